# revision 8
# baseline (speedup 1.0000x reference)
"""Bahdanau attention (with coverage) Trainium2 kernel.

Problem (fp32):
    h_proj   = hidden @ W_h + b_h                     # (B, A)
    img_proj = img @ W_img + b_img                    # (B, N, A)
    cov_proj = coverage[..., None] * W_cov[0]         # (B, N, A)
    energy   = tanh(h_proj[:, None] + img_proj + cov_proj)
    scores   = energy @ v                             # (B, N)
    alpha    = softmax(scores, axis=1)
    context  = sum_n alpha * img                      # (B, H)
    returns (context, alpha)

B=2048, N=49, H=1024, A=512. Data-parallel over 8 cores (256 batches each).

Per-core schedule (token-major, tokens = 256*49 = 12544 = 98 tiles of 128;
2 groups of 128 batches = 49 tiles exactly, since 49*128 tokens = 128 batches):
  - img loaded HBM fp32 -> SBUF bf16 via SWDGE cast-DMA (natural [tok, H]
    tiles, kept resident for the whole group for the context matmul)
  - per 128-token tile: xbar DMA-transpose SBUF->SBUF gives [H-chunk, tok]
    bf16 tiles; PSUM accumulates:
      8x  matmul(imgT_chunk, W_img_chunk)            (bf16)
      1x  matmul(S4 selector, h_proj window)         (folds h_proj + biases)
      1x  matmul(cov row,     W_cov row)             (rank-1 coverage term)
    tanh on ScalarE drains PSUM -> SBUF bf16
  - scores via fused multiply-reduce against partition-replicated v (DVE)
  - per group: scores PE-transposed and bounced through HBM to batch-major
    [128, 49]; softmax; alpha bounced back to token-major
  - context: per tile a [tok, batch] alpha-placement matrix is built with one
    fused (iota == batch_col) * alpha op; bf16 matmuls accumulate the group's
    [128, 1024] context in PSUM over 49 tiles.
"""

import numpy as np

import concourse.bass as bass
import concourse.tile as tile
from concourse import bacc, mybir
from concourse.bass_utils import run_bass_kernel_spmd
from concourse.masks import make_identity

# Problem constants (hardcoded per the harness contract).
B, N, H, A = 2048, 49, 1024, 512
NCORES = 8
BL = B // NCORES          # 256 batches per core
GB = 128                  # batches per group
CH = H // 128             # 8 contraction chunks
TPG = GB * N // 128       # 49 token tiles per group
TC = 7                    # token tiles per DMA chunk
FP32 = mybir.dt.float32
BF16 = mybir.dt.bfloat16


def _sel_tables(n_groups: int):
    """Host-precomputed selector tables for h_proj fold and context build."""
    nt = n_groups * TPG
    t = np.arange(nt * 128)
    tb = t // N                      # global batch per token
    tbl = tb % GB                    # batch local to its group
    j = np.arange(nt)
    b0 = (128 * j) // N              # global batch base per tile
    bl0 = b0 - (j // TPG) * GB       # local base
    blc = np.minimum(bl0, GB - 4)    # clamped so the 4-row window stays in range

    sgrp = np.zeros((128, nt, 128), np.float32)
    for jj in range(nt):
        sgrp[tbl[jj * 128 : (jj + 1) * 128], jj, np.arange(128)] = 1.0

    colb = tbl.reshape(nt, 128).T.astype(np.float32)   # [128, nt]
    iota = np.broadcast_to(np.arange(128, dtype=np.float32), (128, 128)).copy()
    return sgrp, colb, iota, blc


def build_program(n_groups: int = 2, n_devices: int = NCORES):
    """Build the single-core Bass program (SPMD across cores)."""
    ng = n_groups
    nt = ng * TPG                  # token tiles
    bl = ng * GB                   # batches per core
    T = bl * N                     # tokens per core

    nc = bacc.Bacc(
        "TRN2", target_bir_lowering=False, debug=False, num_devices=n_devices
    )

    # --- DRAM I/O ---
    img_d = nc.dram_tensor("img", [T, H], FP32, kind="ExternalInput").ap()
    hT_d = nc.dram_tensor("hiddenT", [H, bl], FP32, kind="ExternalInput").ap()
    cov_d = nc.dram_tensor("cov", [1, T], FP32, kind="ExternalInput").ap()
    wimg_d = nc.dram_tensor("w_img", [128, CH, A], BF16, kind="ExternalInput").ap()
    wh_d = nc.dram_tensor("w_h", [128, CH, A], BF16, kind="ExternalInput").ap()
    bsum_d = nc.dram_tensor("bsum", [1, A], BF16, kind="ExternalInput").ap()
    ones_d = nc.dram_tensor("ones1", [1, 128], BF16, kind="ExternalInput").ap()
    wcov_d = nc.dram_tensor("w_cov", [1, A], BF16, kind="ExternalInput").ap()
    v_d = nc.dram_tensor("v_bc", [128, A], BF16, kind="ExternalInput").ap()
    sgrp_d = nc.dram_tensor("s_grp", [128, nt, 128], BF16, kind="ExternalInput").ap()
    colb_d = nc.dram_tensor("colb", [128, nt], FP32, kind="ExternalInput").ap()
    iota_d = nc.dram_tensor("iota", [128, 128], BF16, kind="ExternalInput").ap()

    ctx_d = nc.dram_tensor("ctx_out", [bl, H], FP32, kind="ExternalOutput").ap()
    alpha_d = nc.dram_tensor("alpha_out", [bl, N], FP32, kind="ExternalOutput").ap()

    scflat_d = nc.dram_tensor("scflat", [nt, 128], FP32).ap()
    alflat_d = nc.dram_tensor("alflat", [nt, 128], FP32).ap()

    with tile.TileContext(nc) as tc:
        with (
            tc.tile_pool(name="const", bufs=1) as const,
            tc.tile_pool(name="nat", bufs=min(8, ng * 7 + 1)) as natp,
            tc.tile_pool(name="tposed", bufs=4) as tp,
            tc.tile_pool(name="energy", bufs=3) as ep,
            tc.tile_pool(name="prod", bufs=2) as prp,
            tc.tile_pool(name="abig", bufs=3) as abp,
            tc.tile_pool(name="covp", bufs=3) as cvp,
            tc.tile_pool(name="small", bufs=2) as sm,
            tc.tile_pool(name="grp", bufs=1) as grp,
            tc.tile_pool(name="pe", bufs=2, space="PSUM") as pse,
            tc.tile_pool(name="pctx", bufs=2, space="PSUM") as pcx,
            tc.tile_pool(name="psmall", bufs=2, space="PSUM") as psm,
        ):
            # --- constants into SBUF ---
            wimg_sb = const.tile([128, CH, A], BF16)
            nc.sync.dma_start(wimg_sb[:], wimg_d[:])
            wh_sb = const.tile([128, CH, A], BF16)
            nc.sync.dma_start(wh_sb[:], wh_d[:])
            bsum_sb = const.tile([1, A], BF16)
            nc.sync.dma_start(bsum_sb[:], bsum_d[:])
            ones_sb = const.tile([1, 128], BF16)
            nc.sync.dma_start(ones_sb[:], ones_d[:])
            wcov_sb = const.tile([1, A], BF16)
            nc.sync.dma_start(wcov_sb[:], wcov_d[:])
            v_sb = const.tile([128, A], BF16)
            nc.sync.dma_start(v_sb[:], v_d[:])
            sgrp_sb = const.tile([128, nt, 128], BF16)
            nc.sync.dma_start(sgrp_sb[:], sgrp_d[:])
            colb_sb = const.tile([128, nt], FP32)
            nc.sync.dma_start(colb_sb[:], colb_d[:])
            iota_sb = const.tile([128, 128], BF16)
            nc.sync.dma_start(iota_sb[:], iota_d[:])
            hT_sb = const.tile([128, CH, bl], BF16)
            nc.gpsimd.dma_start(
                hT_sb[:], hT_d.rearrange("(c p) b -> p c b", p=128)
            )
            ident_sb = const.tile([128, 128], FP32)
            make_identity(nc, ident_sb[:])

            # --- h_proj per group: [128 b, A] = hidden @ W_h + (b_h + b_img) ---
            h_bf = []
            for g in range(ng):
                hp = psm.tile([128, A], FP32, tag='ps_small')
                for c in range(CH):
                    nc.tensor.matmul(
                        hp[:],
                        lhsT=hT_sb[:, c, g * GB : (g + 1) * GB],
                        rhs=wh_sb[:, c, :],
                        start=(c == 0),
                        stop=False,
                    )
                nc.tensor.matmul(
                    hp[:], lhsT=ones_sb[:], rhs=bsum_sb[:], start=False, stop=True
                )
                hbg = grp.tile([128, A], BF16, tag=f"hbf{g}")
                nc.scalar.activation(
                    hbg[:], hp[:], mybir.ActivationFunctionType.Copy
                )
                h_bf.append(hbg)

            # --- main loop ---
            for g in range(ng):
                scores_g = grp.tile([128, TPG], FP32, tag=f"scores{g}")
                nats = []
                for jc in range(7):           # 7 chunks of 7 tiles per group
                    chunk0 = (g * 7 + jc) * TC * 128
                    nat = natp.tile([128, TC, H], BF16)
                    nats.append(nat)
                    cov_ch = cvp.tile([1, TC * 128], BF16, tag="cov")
                    nc.gpsimd.dma_start(
                        cov_ch[:], cov_d[:, chunk0 : chunk0 + TC * 128]
                    )
                    nc.gpsimd.dma_start(
                        nat[:],
                        img_d[chunk0 : chunk0 + TC * 128, :].rearrange(
                            "(t p) h -> p t h", p=128
                        ),
                    )
                    for t in range(TC):
                        j = (g * 7 + jc) * TC + t
                        jj = jc * TC + t      # tile index within group
                        imgT = tp.tile([128, CH, 128], BF16)
                        nc.sync.dma_start_transpose(imgT[:], nat[:, t, :])
                        eps = pse.tile([128, A], FP32)
                        for c in range(CH):
                            nc.tensor.matmul(
                                eps[:],
                                lhsT=imgT[:, c, :],
                                rhs=wimg_sb[:, c, :],
                                start=(c == 0),
                                stop=False,
                            )
                        nc.tensor.matmul(
                            eps[:],
                            lhsT=sgrp_sb[:, j, :],
                            rhs=h_bf[g][:],
                            start=False,
                            stop=False,
                        )
                        nc.tensor.matmul(
                            eps[:],
                            lhsT=cov_ch[:, t * 128 : (t + 1) * 128],
                            rhs=wcov_sb[:],
                            start=False,
                            stop=True,
                        )
                        en = ep.tile([128, A], BF16)
                        nc.scalar.activation(
                            en[:], eps[:], mybir.ActivationFunctionType.Tanh
                        )
                        pr = prp.tile([128, A], BF16)
                        nc.vector.tensor_mul(pr[:], en[:], v_sb[:])
                        nc.vector.tensor_reduce(
                            scores_g[:, jj : jj + 1],
                            pr[:],
                            axis=mybir.AxisListType.X,
                            op=mybir.AluOpType.add,
                        )

                # --- softmax (batch-major bounce) ---
                scT_ps = psm.tile([TPG, 128], FP32, tag='ps_small')
                nc.tensor.transpose(scT_ps[:], scores_g[:], ident_sb[:])
                scT = sm.tile([TPG, 128], FP32)
                nc.vector.tensor_copy(scT[:], scT_ps[:])
                nc.sync.dma_start(scflat_d[g * TPG : (g + 1) * TPG, :], scT[:])
                sc_bm = sm.tile([128, N], FP32)
                nc.sync.dma_start(
                    sc_bm[:],
                    scflat_d.rearrange("j p -> (j p)")[
                        g * TPG * 128 : (g + 1) * TPG * 128
                    ].rearrange("(b n) -> b n", n=N),
                )
                mx = sm.tile([128, 1], FP32)
                nc.vector.tensor_reduce(
                    mx[:],
                    sc_bm[:],
                    axis=mybir.AxisListType.X,
                    op=mybir.AluOpType.max,
                    negate=True,
                )
                ex = sm.tile([128, N], FP32)
                sume = sm.tile([128, 1], FP32)
                nc.scalar.activation(
                    ex[:],
                    sc_bm[:],
                    mybir.ActivationFunctionType.Exp,
                    bias=mx[:],
                    accum_out=sume[:],
                )
                rc = sm.tile([128, 1], FP32)
                nc.vector.reciprocal(rc[:], sume[:])
                al_bm = sm.tile([128, N], FP32)
                nc.vector.tensor_scalar_mul(al_bm[:], ex[:], rc[:])
                nc.sync.dma_start(alpha_d[g * GB : (g + 1) * GB, :], al_bm[:])
                nc.sync.dma_start(
                    alflat_d.rearrange("j p -> (j p)")[
                        g * TPG * 128 : (g + 1) * TPG * 128
                    ].rearrange("(b n) -> b n", n=N),
                    al_bm[:],
                )
                alT = sm.tile([TPG, 128], FP32)
                nc.sync.dma_start(alT[:], alflat_d[g * TPG : (g + 1) * TPG, :])
                aT_ps = psm.tile([128, TPG], FP32, tag='ps_small')
                nc.tensor.matmul(
                    aT_ps[:],
                    lhsT=alT[:],
                    rhs=ident_sb[:TPG, :TPG],
                    is_transpose=True,
                    start=True,
                    stop=True,
                )
                a_all = sm.tile([128, TPG], FP32)
                nc.vector.tensor_copy(a_all[:], aT_ps[:])

                # --- context: ctx[b, h] = sum_t alpha[t] img[t, h] ---
                cps = pcx.tile([128, H], FP32)
                for jj in range(TPG):
                    j = g * TPG + jj
                    ab = abp.tile([128, 128], BF16)
                    nc.vector.tensor_scalar(
                        ab[:],
                        iota_sb[:],
                        colb_sb[:, j : j + 1],
                        a_all[:, jj : jj + 1],
                        op0=mybir.AluOpType.is_equal,
                        op1=mybir.AluOpType.mult,
                    )
                    nat = nats[jj // TC]
                    for half in range(2):
                        nc.tensor.matmul(
                            cps[:, half * 512 : (half + 1) * 512],
                            lhsT=ab[:],
                            rhs=nat[:, jj % TC, half * 512 : (half + 1) * 512],
                            start=(jj == 0),
                            stop=(jj == TPG - 1),
                        )
                ctx_sb = sm.tile([128, H], FP32, bufs=1)
                nc.vector.tensor_copy(ctx_sb[:], cps[:])
                nc.sync.dma_start(ctx_d[g * GB : (g + 1) * GB, :], ctx_sb[:])

    nc.compile()
    return nc


def make_in_maps(hidden, img_features, coverage, W_h, b_h, W_img, b_img, W_cov, v,
                 n_groups: int = 2, n_cores: int = NCORES):
    """Shard + lay out inputs for the SPMD program."""
    ng = n_groups
    bl = ng * GB
    T = bl * N
    sgrp, colb, iota, _ = _sel_tables(ng)

    import ml_dtypes

    bf = ml_dtypes.bfloat16
    consts = {
        "w_img": np.ascontiguousarray(
            W_img.reshape(CH, 128, A).transpose(1, 0, 2)
        ).astype(bf),
        "w_h": np.ascontiguousarray(
            W_h.reshape(CH, 128, A).transpose(1, 0, 2)
        ).astype(bf),
        "bsum": (b_h + b_img).reshape(1, A).astype(bf),
        "ones1": np.ones((1, 128), bf),
        "w_cov": W_cov.reshape(1, A).astype(bf),
        "v_bc": np.broadcast_to(v, (128, A)).astype(bf),
        "s_grp": sgrp.astype(bf),
        "colb": colb.astype(np.float32),
        "iota": iota.astype(bf),
    }
    in_maps = []
    for i in range(n_cores):
        b0, b1 = i * bl, (i + 1) * bl
        m = dict(consts)
        m["img"] = np.ascontiguousarray(
            img_features[b0:b1].reshape(T, H)
        ).astype(np.float32)
        m["hiddenT"] = np.ascontiguousarray(hidden[b0:b1].T).astype(np.float32)
        m["cov"] = np.ascontiguousarray(coverage[b0:b1].reshape(1, T)).astype(
            np.float32
        )
        in_maps.append(m)
    return in_maps


_NC_CACHE = {}


def kernel(hidden, img_features, coverage, W_h, b_h, W_img, b_img, W_cov, v):
    hidden = np.asarray(hidden, np.float32)
    img_features = np.asarray(img_features, np.float32)
    coverage = np.asarray(coverage, np.float32)
    W_h = np.asarray(W_h, np.float32)
    b_h = np.asarray(b_h, np.float32)
    W_img = np.asarray(W_img, np.float32)
    b_img = np.asarray(b_img, np.float32)
    W_cov = np.asarray(W_cov, np.float32)
    v = np.asarray(v, np.float32)

    if "nc" not in _NC_CACHE:
        _NC_CACHE["nc"] = build_program(n_groups=2, n_devices=NCORES)
    nc = _NC_CACHE["nc"]

    in_maps = make_in_maps(
        hidden, img_features, coverage, W_h, b_h, W_img, b_img, W_cov, v
    )
    res = run_bass_kernel_spmd(
        nc, in_maps, core_ids=list(range(NCORES)), trace=False
    ).results
    context = np.concatenate([res[i]["ctx_out"] for i in range(NCORES)], axis=0)
    alpha = np.concatenate([res[i]["alpha_out"] for i in range(NCORES)], axis=0)
    return (context.astype(np.float32), alpha.astype(np.float32))
